# revision 14
# baseline (speedup 1.0000x reference)
"""Expert-parallel MoE routing kernel for Trainium2 (8 NeuronCores).

Problem: out[t] = x[t] @ W[idx[t]].T + b[idx[t]],  idx = pointer_addresses % 8
  x: [2048, 512] f32, W: [8, 8192, 512] f32, b: [8, 8192] f32 -> out [2048, 8192] f32

Strategy: expert parallel. Host computes idx, gathers each expert's tokens
(padded to a common capacity `cap`), and each core e computes
  out_e = x_e @ W[e].T + b[e]
with the vocab dimension on PSUM partitions so the bias is a fused
per-partition bias on the Scalar/Vector engines. Host scatters rows back.

Per-core matmul orientation (out = lhsT.T @ rhs):
  lhsT = W chunk  [K=128 (d inner), M=128 (vocab cols)]   (stationary)
  rhs  = xT chunk [K=128 (d inner), N=cap (tokens)]        (moving)
  psum [128 vocab, cap tokens] accumulated over 4 K-chunks of D=512.

The 64 vocab chunks are streamed in groups; group sizes are graduated
(small first/last) so the serial prologue (first W load) and epilogue
(last out store) are short while steady-state DMAs stay large.
"""

import os

import numpy as np

E = 8          # experts == cores
D = 512        # hidden
V = 8192       # out features
P = 128        # partitions
KCH = D // P   # 4 contraction chunks
VCH = V // P   # 64 vocab chunks

# matmul input dtype: 'f32' (exact, 4 cyc/row), 'f32r' (~full speed, ~1.3e-4
# rel err), 'fp16'/'bf16' (full speed, ~2.6e-4 / ~2.1e-3 rel err)
MM_DTYPE = os.environ.get("KERNEL_MM_DTYPE", "fp16")
# output storage dtype: 'f32' (exact) | 'fp16' (~2.4e-4 quant err, halves out bytes)
OUT_DTYPE = os.environ.get("KERNEL_OUT_DTYPE", "fp16")
_GROUPS_ENV = os.environ.get("KERNEL_GROUPS")
# microbench variants: 'full' | 'dmaonly' (DMAs, no compute) | 'computeonly'
# (compute from one resident W buffer, no steady-state DMA) | 'wonly'
# (W-load DMAs only) | 'mmonly' (matmuls only, no evictions/out)
VARIANT = os.environ.get("KERNEL_VARIANT", "full")

LAST_RESULT = None  # BassKernelResults of the most recent run (for test harness)

_BUILD_CACHE = {}


def _in_sz():
    return 2 if MM_DTYPE in ("bf16", "fp16") else 4


def _out_sz():
    return 2 if OUT_DTYPE in ("bf16", "fp16") else 4


def _base_gv(cap):
    """Steady-state vocab chunks per DMA group: as large as SBUF allows.

    Per-partition slab budgets: w tiles gv*KCH*P*in_sz (x3 bufs), o tiles
    gv*cap*out_sz (x2 bufs), plus the resident x tile. 16 fits at the
    nominal cap (~274); shrink for pathologically imbalanced routing.
    """
    for gv in (16, 8, 4, 2, 1):
        if (
            gv * KCH * P * _in_sz() * 3
            + gv * cap * _out_sz() * 2
            + KCH * cap * _in_sz()
            <= 168 * 1024
        ):
            return gv
    return 1


def _groups(cap):
    """Graduated group schedule over the 64 vocab chunks.

    Small first groups so compute starts as soon as ~2 chunks of W land
    (instead of waiting for a full-size load); small last group so the
    serial epilogue (last out store after last compute) is short. Steady
    state uses full-size groups for DMA efficiency.
    """
    if _GROUPS_ENV:
        sched = [int(v) for v in _GROUPS_ENV.split(",")]
        assert sum(sched) == VCH
        return sched
    base = _base_gv(cap)
    if base == 16:
        sched = [2, 4, 8, 16, 16, 10, 6, 2]
    else:
        sched = []
        left = VCH
        while left > 0:
            g = min(base, left)
            sched.append(g)
            left -= g
    assert sum(sched) == VCH
    return sched


def _build(cap, repeat=1, loop_n=1):
    """Build the per-core Bass module for token capacity `cap`.

    repeat/loop_n > 1 re-run the compute loop (same outputs) so the test
    harness can difference wall-times to isolate on-device kernel time;
    loop_n uses a hardware For_i loop (constant code size).
    """
    key = (cap, MM_DTYPE, OUT_DTYPE, tuple(_groups(cap)), repeat, loop_n, VARIANT)
    if key in _BUILD_CACHE:
        return _BUILD_CACHE[key]

    import concourse.mybir as mybir
    from concourse import bacc
    from concourse.tile import TileContext

    dt_in = {
        "f32": mybir.dt.float32,
        "f32r": mybir.dt.float32r,
        "bf16": mybir.dt.bfloat16,
        "fp16": mybir.dt.float16,
    }[MM_DTYPE]
    f32 = mybir.dt.float32
    out_dt = {"f32": f32, "fp16": mybir.dt.float16, "bf16": mybir.dt.bfloat16}[
        OUT_DTYPE
    ]
    assert cap <= 512, f"psum slot scheme needs cap<=512, got {cap}"

    nc = bacc.Bacc(None, target_bir_lowering=False)
    # flat layouts, vocab-chunk (vi) as the per-partition-contiguous axis
    wt = nc.dram_tensor("wt", [P, VCH, KCH, P], dt_in, kind="ExternalInput")
    xt = nc.dram_tensor("xt", [P, KCH, cap], dt_in, kind="ExternalInput")
    bias = nc.dram_tensor("bias", [P, VCH], f32, kind="ExternalInput")
    out = nc.dram_tensor("out", [P, VCH, cap], out_dt, kind="ExternalOutput")

    # W-load completion granularity (chunks per dma_start): small first
    # groups so compute starts early; all ride the SP HWDGE ring in order.
    W_SCHED = [2, 4] + [8] * 7 + [2]
    assert sum(W_SCHED) == VCH
    # out-store granularity; last group small to shrink the serial epilogue
    OUT_SCHED = [8] * 7 + [6, 2]
    assert sum(OUT_SCHED) == VCH and all(g % 2 == 0 for g in OUT_SCHED)

    do_w_dma = VARIANT in ("full", "dmaonly", "wonly")
    do_compute = VARIANT in ("full", "computeonly", "mmonly")
    do_evict = VARIANT in ("full", "computeonly")
    do_out = VARIANT in ("full", "dmaonly")

    def group_body():
        if VARIANT.startswith("wbig"):
            if VARIANT == "wbig1":
                nc.sync.dma_start(w_all, wt.ap())
            else:
                h = VCH // 2
                nc.sync.dma_start(w_all[:, :h], wt.ap()[:, :h])
                nc.scalar.dma_start(w_all[:, h:], wt.ap()[:, h:])
            return
        # issue the whole W-load stream up front; the permanent buffer has
        # no rotation hazards, and subtile deps release compute per group
        if do_w_dma:
            vi0 = 0
            for nvi in W_SCHED:
                nc.sync.dma_start(
                    w_all[:, vi0 : vi0 + nvi], wt.ap()[:, vi0 : vi0 + nvi]
                )
                vi0 += nvi
        elif do_compute:
            # compute-only benches run from the first W group alone
            nc.sync.dma_start(w_all[:, : W_SCHED[0]], wt.ap()[:, : W_SCHED[0]])
        if do_compute:
            for vi in range(0, VCH, 2):
                # dual-chunk PSUM tile: each 512-col f32 slot is exactly one
                # 2KB bank, so matmul writes stay bank-local
                ps = pp.tile([P, 2, 512], f32, tag="ps")
                for c in range(2):
                    wv = vi + c if do_w_dma else (vi + c) % W_SCHED[0]
                    for k in range(KCH):
                        nc.tensor.matmul(
                            ps[:, c, :cap],
                            lhsT=w_all[:, wv, k],
                            rhs=x_sb[:, k],
                            start=(k == 0),
                            stop=(k == KCH - 1),
                        )
                if do_evict:
                    # out = psum + bias for both chunks in one DVE op;
                    # ScalarE stays free for DMA issue only
                    nc.vector.tensor_tensor(
                        o_all[:, vi : vi + 2],
                        ps[:, :, :cap],
                        b_sb[:, vi : vi + 2].to_broadcast((P, 2, cap)),
                        mybir.AluOpType.add,
                    )
        if do_out:
            vi0 = 0
            for nvi in OUT_SCHED:
                nc.scalar.dma_start(
                    out.ap()[:, vi0 : vi0 + nvi], o_all[:, vi0 : vi0 + nvi]
                )
                vi0 += nvi

    with TileContext(nc) as tc:
        with (
            tc.tile_pool(name="perm", bufs=1) as perm,
            tc.tile_pool(name="pp", bufs=4, space="PSUM") as pp,
        ):
            # x/bias ride the ACT HWDGE ring (fast first-byte, parallel with
            # the first W group on the SP ring) so compute can start early
            x_sb = perm.tile([P, KCH, cap], dt_in, name="x_sb")
            nc.scalar.dma_start(x_sb, xt.ap())
            b_sb = perm.tile([P, VCH], f32, name="b_sb")
            nc.scalar.dma_start(b_sb, bias.ap())
            w_all = perm.tile([P, VCH, KCH, P], dt_in, name="w_all")
            o_all = None
            if do_evict or do_out:
                o_all = perm.tile([P, VCH, cap], out_dt, name="o_all")
            if VARIANT == "dmaonly":
                nc.vector.memset(o_all, 0.0)

            import contextlib

            loop_cm = (
                tc.For_i(0, loop_n, 1) if loop_n > 1 else contextlib.nullcontext()
            )
            with loop_cm:
                for _rep in range(repeat):
                    group_body()

    nc.finalize()
    _BUILD_CACHE[key] = nc
    return nc


def _prepare(x, pointer_addresses, W, b):
    """Host-side shard: gather tokens per expert, lay out per-core inputs."""
    x = np.ascontiguousarray(np.asarray(x), dtype=np.float32)
    W = np.ascontiguousarray(np.asarray(W), dtype=np.float32)
    b = np.ascontiguousarray(np.asarray(b), dtype=np.float32)
    pa = np.asarray(pointer_addresses)

    idx = (pa.astype(np.int64) % E).astype(np.int64)
    rows = [np.flatnonzero(idx == e) for e in range(E)]
    counts = np.array([len(r) for r in rows])
    cap = max(256, int(counts.max()))

    if MM_DTYPE == "bf16":
        import ml_dtypes

        np_dt = np.dtype(ml_dtypes.bfloat16)
    elif MM_DTYPE == "fp16":
        np_dt = np.dtype(np.float16)
    else:
        np_dt = np.dtype(np.float32)

    in_maps = []
    for e in range(E):
        # xT: [P(d inner), KCH, cap]
        x_pad = np.zeros((cap, D), np.float32)
        x_pad[: counts[e]] = x[rows[e]]
        xt_e = np.ascontiguousarray(
            x_pad.reshape(cap, KCH, P).transpose(2, 1, 0).astype(np_dt)
        )
        # wt: [p, vi, k, c] = W[e, vi*P + c, k*P + p]
        w_e = np.ascontiguousarray(
            W[e].reshape(VCH, P, KCH, P).transpose(3, 0, 2, 1).astype(np_dt)
        )
        # bias: [P(c), VCH]
        b_e = np.ascontiguousarray(b[e].reshape(VCH, P).T)
        in_maps.append({"wt": w_e, "xt": xt_e, "bias": b_e})

    return in_maps, rows, counts, cap


def _run(nc, in_maps):
    global LAST_RESULT
    from concourse.bass_utils import run_bass_kernel_spmd

    res = run_bass_kernel_spmd(nc, in_maps, core_ids=list(range(E)))
    LAST_RESULT = res
    return res


def _assemble(res, rows, counts, cap, n_tokens):
    out = np.zeros((n_tokens, V), np.float32)
    for e in range(E):
        # out dram [P(c), VCH, cap] -> vocab-major [V, cap]
        o = (
            res.results[e]["out"]
            .astype(np.float32)
            .transpose(1, 0, 2)
            .reshape(V, cap)
        )
        out[rows[e]] = o[:, : counts[e]].T
    return out


def kernel(x, pointer_addresses, W, b):
    in_maps, rows, counts, cap = _prepare(x, pointer_addresses, W, b)
    nc = _build(cap)
    res = _run(nc, in_maps)
    return _assemble(res, rows, counts, cap, np.asarray(x).shape[0])

